# revision 1
# baseline (speedup 1.0000x reference)
"""Trainium2 Bass kernel: per-batch grouped Conv2d (16 batches, 1->32 ch, 9x9, pad=3).

Pure data parallel: 2 batches per core on 8 NeuronCores.  Per batch:
  out[j, y, x] = sum_{ky,kx} W[j,ky,kx] * xpad[y+ky, x+kx]
computed as 9 PSUM-accumulated matmuls (one per kernel column dx) with
contraction over 12 padded input rows (K=12).  One output block = 4 output
rows x 32 channels = 128 PSUM partitions x 510 columns.  Four PE row-strips
(tile_position (0|32|64|96, 0)) run 4 blocks concurrently; fp32r matmuls
stream at ~1 col/cycle.  DVE/ACT evacuate PSUM -> SBUF, HWDGE DMA stores.
"""

import numpy as np

import concourse.bacc as bacc
import concourse.mybir as mybir
from concourse.bass_utils import run_bass_kernel_spmd
from concourse.tile import TileContext

B, J, KH, KW = 16, 32, 9, 9
H = W_IN = 512
PAD = 3          # int(9/2) - 1
HO = WO = 510    # 512 + 2*3 - 9 + 1
NCORES = 8
BPC = B // NCORES          # batches per core = 2
XP = 520                   # padded row length: 3 + 512 + 5
XR = 524                   # padded rows: 3 + 512 + 9 (strip slicing headroom)
NROUND = 32                # 32 rounds x 4 strips x 4 rows = 512 out rows (last 2 dropped)

DT = mybir.dt.float32
DTR = mybir.dt.float32r

_PROG_CACHE = {}


def _build_program(repeat=1, timing=False):
    nc = bacc.Bacc("TRN2", target_bir_lowering=False, debug=False,
                   num_devices=NCORES)
    xpad = nc.dram_tensor("xpad", [BPC, XR, XP], DTR, kind="ExternalInput")
    wprep = nc.dram_tensor("wprep", [BPC, 12, KW, 128], DTR, kind="ExternalInput")
    if timing:
        # timing build: full-size result stays in device DRAM; only a tiny
        # tensor is transferred back, so wall-clock deltas isolate HW time.
        out = nc.dram_tensor("out_scratch", [BPC, J, HO, WO], DT)
        dummy = nc.dram_tensor("tdummy", [1, 128], DT, kind="ExternalOutput")
    else:
        out = nc.dram_tensor("out", [BPC, J, HO, WO], DT, kind="ExternalOutput")

    with TileContext(nc) as tc:
        with (
            tc.tile_pool(name="wpool", bufs=1) as wpool,
            tc.tile_pool(name="imgpool", bufs=2) as imgpool,
            tc.tile_pool(name="pspool", bufs=2, space="PSUM") as pspool,
            tc.tile_pool(name="evpool", bufs=3) as evpool,
            tc.tile_pool(name="scrpool", bufs=2, space="DRAM") as scrpool,
        ):
            # Stationary weight tiles, replicated on all 4 strips.
            # Per strip s (partitions 32s..32s+11):
            #   free [b*1152 + dx*128 + m] = wprep[b, dy', dx, m]
            wt = wpool.tile([128, BPC * KW * 128], DTR)
            for s in range(4):
                for b in range(BPC):
                    nc.sync.dma_start(
                        out=wt[32 * s:32 * s + 12,
                               b * KW * 128:(b + 1) * KW * 128],
                        in_=wprep[b].rearrange("p a m -> p (a m)"),
                    )

            for _ in range(repeat):
                for b in range(BPC):
                    # Image row panels: strip s, slot j holds padded rows
                    # 16j+4s+dy' (dy'=0..11) at free offset j*520.
                    img = imgpool.tile([128, NROUND * XP], DTR)
                    for s in range(4):
                        src = xpad[b, 4 * s:4 * s + 512, :] \
                            .rearrange("(j p) x -> p j x", p=16)[0:12]
                        nc.sync.dma_start(
                            out=img[32 * s:32 * s + 12, :]
                                .rearrange("p (j x) -> p j x", x=XP),
                            in_=src,
                        )

                    for j in range(NROUND):
                        pss = [pspool.tile([128, WO], DT, tag=f"ps{s}",
                                           name=f"ps{s}_{b}_{j}")
                               for s in range(4)]
                        for dx in range(KW):
                            for s in range(4):
                                lhsT = wt[32 * s:32 * s + 12,
                                          b * KW * 128 + dx * 128:
                                          b * KW * 128 + (dx + 1) * 128]
                                rhs = img[32 * s:32 * s + 12,
                                          j * XP + dx:j * XP + dx + WO]
                                nc.tensor.matmul(
                                    pss[s][:], lhsT, rhs,
                                    start=(dx == 0), stop=(dx == KW - 1),
                                    tile_position=(32 * s, 0),
                                )
                        ev = evpool.tile([128, 4 * WO], DT)
                        for s in range(4):
                            if s == 3:
                                nc.scalar.copy(ev[:, s * WO:(s + 1) * WO],
                                               pss[s][:])
                            else:
                                nc.vector.tensor_copy(ev[:, s * WO:(s + 1) * WO],
                                                      pss[s][:])
                        # store: rows 16j+4s .. +3; partition p = ch*4 + sy.
                        # src stays [128, 510]; the DMA balancer splits the
                        # partition dim against the [32, 4, 510] DRAM dest.
                        for s in range(4):
                            src2 = ev[:, s * WO:(s + 1) * WO]
                            if j < NROUND - 1 or s < 3:
                                nc.sync.dma_start(
                                    out=out[b, :, 16 * j + 4 * s:
                                            16 * j + 4 * s + 4, :],
                                    in_=src2,
                                )
                            else:
                                # block (31,3) covers rows 508..511; keep
                                # 508/509 via DRAM bounce (sy-subset of the
                                # partition dim is not a rectangular AP).
                                scr = scrpool.tile([J, 4, WO], DT)
                                nc.sync.dma_start(out=scr[:], in_=src2)
                                nc.gpsimd.dma_start(
                                    out=out[b, :, 508:510, :],
                                    in_=scr[:, 0:2, :],
                                )
            if timing:
                nc.sync.dma_start(out=dummy[:], in_=wt[0:1, 0:128].bitcast(DT))
    nc.compile()
    return nc


def _get_program(repeat=1, timing=False):
    key = (repeat, timing)
    if key not in _PROG_CACHE:
        _PROG_CACHE[key] = _build_program(repeat, timing)
    return _PROG_CACHE[key]


def _prep_core_inputs(input, weight, c):
    xp = np.zeros((BPC, XR, XP), np.float32)
    xp[:, PAD:PAD + H, PAD:PAD + W_IN] = input[BPC * c:BPC * (c + 1), 0]
    wp = np.zeros((BPC, 12, KW, 128), np.float32)
    wsl = weight[BPC * c:BPC * (c + 1)]            # [2, 32, 9, 9]
    wq = wsl.transpose(0, 2, 3, 1)                 # [2, ky, kx, j]
    for sy in range(4):
        # wp[b, sy+ky, dx, j*4+sy] = W[b, j, ky, dx]
        wp[:, sy:sy + 9, :, sy::4] = wq
    return {"xpad": xp, "wprep": wp}


def kernel(input, weight, _repeat=1, _timing=False):
    input = np.ascontiguousarray(np.asarray(input, np.float32))
    weight = np.ascontiguousarray(np.asarray(weight, np.float32))
    nc = _get_program(_repeat, _timing)
    in_maps = [_prep_core_inputs(input, weight, c) for c in range(NCORES)]
    res = run_bass_kernel_spmd(nc, in_maps, list(range(NCORES)))
    if _timing:
        return None
    outs = np.stack([res.results[c]["out"] for c in range(NCORES)])
    return outs.reshape(B, J, HO, WO).astype(np.float32, copy=False)

